# revision 5
# baseline (speedup 1.0000x reference)
"""Trainium2 Bass kernel for nn_AsymmetricLossCustomMS.

Reference math per sample b (x, y, y_neg: [B, C]; group_mask: [L, C]):
  xs     = sigmoid(x)
  thres  = max(16th-largest of xs, 0.3)
  gmax_l = max over classes in group l of xs        (L groups)
  gt_l   = any positive y in group l; gt_neg_l likewise for y_neg
  caseB  = sum_l rank_loss picked by gt_l           (if any gt_l)
  caseA  = mix of union-max and neg-score rank losses (otherwise)
  loss   = mean over b

Strategy: pure data parallel over the batch (256 rows/core on 8 cores).
sigmoid is monotonic, so the 16th-largest and group maxima are computed on
raw x and sigmoided afterwards (tiny [128, L] tensors). Only classes that
belong to some whitelist group matter for y/y_neg/group-max, so the host
gathers those columns into a compact padded [3, L, W] layout ("z") where
every per-group segment is a contiguous width-W slice; all 3*L segment
reductions then run as ONE vector reduce per row-tile. The 16th-largest
uses the DVE top-8 instruction pair: max -> match_replace -> max.
"""

import numpy as np

B, C, L = 2048, 9605, 8
N_CORES = 8
ROWS = B // N_CORES  # 256 rows per core
P = 128              # SBUF partitions per row-tile
TILES = ROWS // P    # 2 row-tiles per core
C_PAD = 9608         # x padded to a multiple of 8 columns
NEG = -1e30
ALPHA = 0.5    # caseA mix
ALPHA1 = 0.05  # margin
ALPHA3 = 5.0   # logistic sharpness
ALPHA_OTHER = 0.3
TOPK = 16

USE_BF16 = True
STAGE = 99  # truncate the per-tile program after this stage (debug aid)

LAST_RESULT = None  # BassKernelResults of the most recent run (for test harness)

_graph_cache = {}


def _build(W):
    import concourse.bacc as bacc
    import concourse.tile as tile
    from concourse import mybir
    from concourse.alu_op_type import AluOpType as Op

    DT = mybir.dt.bfloat16 if USE_BF16 else mybir.dt.float32
    F32 = mybir.dt.float32
    SIG = mybir.ActivationFunctionType.Sigmoid
    X = mybir.AxisListType.X
    ZW = 3 * L * W

    nc = bacc.Bacc("TRN2", target_bir_lowering=False, debug=False, num_devices=N_CORES)
    x_d = nc.dram_tensor("x", [ROWS, C_PAD], DT, kind="ExternalInput")
    z_d = nc.dram_tensor("z", [ROWS, ZW], DT, kind="ExternalInput")
    out_d = nc.dram_tensor("loss", [TILES, P], F32, kind="ExternalOutput")

    with tile.TileContext(nc) as tc:
        with tc.tile_pool(name="consts", bufs=1) as consts, \
             tc.tile_pool(name="big", bufs=2) as big, \
             tc.tile_pool(name="med", bufs=2) as med, \
             tc.tile_pool(name="small", bufs=2) as small:
            bias_c = consts.tile([P, 1], F32)
            nc.vector.memset(bias_c, ALPHA3 * ALPHA1)
            for t in range(TILES):
                r0 = t * P
                xt = big.tile([P, C_PAD], DT)
                nc.gpsimd.dma_start(out=xt, in_=x_d.ap()[r0:r0 + P, :])
                zt = med.tile([P, 3 * L, W], DT)
                nc.gpsimd.dma_start(
                    out=zt,
                    in_=z_d.ap()[r0:r0 + P, :].rearrange("p (g w) -> p g w", w=W),
                )

                if STAGE < 1:
                    nc.gpsimd.dma_start(out=out_d.ap()[t:t + 1, :], in_=xt[:, 0:1])
                    continue
                # 16th largest of x row: top-8, zap them, top-8 again.
                t8a = small.tile([P, 8], DT)
                nc.vector.max(out=t8a, in_=xt)
                nc.vector.match_replace(
                    out=xt, in_to_replace=t8a, in_values=xt, imm_value=NEG
                )
                t8b = small.tile([P, 8], DT)
                nc.vector.max(out=t8b, in_=xt)
                thres = small.tile([P, 1], F32)
                nc.scalar.activation(out=thres, in_=t8b[:, 7:8], func=SIG)
                nc.vector.tensor_scalar_max(thres, thres, ALPHA_OTHER)

                if STAGE < 2:
                    nc.gpsimd.dma_start(out=out_d.ap()[t:t + 1, :], in_=thres)
                    continue
                # All 24 segment maxima in one reduce: [gmax | gty | gtn].
                red = small.tile([P, 3 * L], F32)
                nc.vector.reduce_max(out=red, in_=zt, axis=X)
                gmax = red[:, 0:L]
                gty = red[:, L:2 * L]      # 1.0 iff group has a positive y
                gtn = red[:, 2 * L:3 * L]  # 1.0 iff group has a positive y_neg

                if STAGE < 3:
                    nc.gpsimd.dma_start(out=out_d.ap()[t:t + 1, :], in_=red[:, 0:1])
                    continue
                gsig = small.tile([P, L], F32)
                nc.scalar.activation(out=gsig, in_=gmax, func=SIG)

                # un[:,0] = union max (= max of group sigmoids)
                # un[:,1] = neg_score (= max_l gtn_l * gsig_l; 0 when no gtn)
                un = small.tile([P, 2], F32)
                nc.vector.reduce_max(out=un[:, 0:1], in_=gsig, axis=X)
                negp = small.tile([P, L], F32)
                nc.vector.tensor_mul(negp, gtn, gsig)
                nc.vector.reduce_max(out=un[:, 1:2], in_=negp, axis=X)

                if STAGE < 4:
                    nc.gpsimd.dma_start(out=out_d.ap()[t:t + 1, :], in_=un[:, 0:1])
                    continue
                # caseB: d_l = (gsig_l - thres) * (1 - 2*gt_l); per-group loss
                # sigmoid(5*d + 0.25) * (1 + (d > -0.05)); summed over l.
                sgn = small.tile([P, L], F32)
                nc.vector.tensor_scalar(
                    out=sgn, in0=gty, scalar1=-2.0, scalar2=1.0,
                    op0=Op.mult, op1=Op.add,
                )
                dm = small.tile([P, L], F32)
                nc.vector.scalar_tensor_tensor(
                    out=dm, in0=gsig, scalar=thres, in1=sgn,
                    op0=Op.subtract, op1=Op.mult,
                )
                sB = small.tile([P, L], F32)
                nc.scalar.activation(
                    out=sB, in_=dm, func=SIG, scale=ALPHA3, bias=bias_c[:]
                )
                pB = small.tile([P, L], F32)
                nc.vector.tensor_scalar(
                    out=pB, in0=dm, scalar1=-ALPHA1, scalar2=1.0,
                    op0=Op.is_gt, op1=Op.add,
                )
                fB = small.tile([P, L], F32)
                nc.vector.tensor_mul(fB, sB, pB)
                caseB = small.tile([P, 1], F32)
                nc.vector.reduce_sum(out=caseB, in_=fB, axis=X)

                if STAGE < 5:
                    nc.gpsimd.dma_start(out=out_d.ap()[t:t + 1, :], in_=caseB)
                    continue
                # caseA on the packed [umax, neg_score] pair.
                dA = small.tile([P, 2], F32)
                nc.vector.tensor_scalar(
                    out=dA, in0=un, scalar1=thres, scalar2=None, op0=Op.subtract
                )
                sA = small.tile([P, 2], F32)
                nc.scalar.activation(
                    out=sA, in_=dA, func=SIG, scale=ALPHA3, bias=bias_c[:]
                )
                pA = small.tile([P, 2], F32)
                nc.vector.tensor_scalar(
                    out=pA, in0=dA, scalar1=-ALPHA1, scalar2=1.0,
                    op0=Op.is_gt, op1=Op.add,
                )
                fA = small.tile([P, 2], F32)
                nc.vector.tensor_mul(fA, sA, pA)
                caseAr = small.tile([P, 1], F32)
                nc.vector.reduce_sum(out=caseAr, in_=fA, axis=X)
                caseA = small.tile([P, 1], F32)
                nc.vector.tensor_scalar(
                    out=caseA, in0=caseAr, scalar1=ALPHA, scalar2=None, op0=Op.mult
                )

                # loss = caseA + has_gt * (caseB - caseA)
                hg = small.tile([P, 1], F32)
                nc.vector.reduce_max(out=hg, in_=gty, axis=X)
                dd = small.tile([P, 1], F32)
                nc.vector.tensor_sub(dd, caseB, caseA)
                nc.vector.tensor_mul(dd, dd, hg)
                lossr = small.tile([P, 1], F32)
                nc.vector.tensor_add(lossr, caseA, dd)
                nc.gpsimd.dma_start(out=out_d.ap()[t:t + 1, :], in_=lossr)
    nc.compile()
    return nc


def kernel(x, y, y_neg, group_mask):
    global LAST_RESULT
    from concourse.bass_utils import run_bass_kernel_spmd

    x = np.asarray(x, dtype=np.float32)
    y = np.asarray(y, dtype=np.float32)
    y_neg = np.asarray(y_neg, dtype=np.float32)
    gm = np.asarray(group_mask).astype(bool)

    if USE_BF16:
        import ml_dtypes

        DT = ml_dtypes.bfloat16
    else:
        DT = np.float32

    cols = [np.flatnonzero(gm[l]) for l in range(L)]
    wmax = max((len(c) for c in cols), default=1)
    W = ((max(wmax, 1) + 7) // 8) * 8

    xp = np.full((B, C_PAD), NEG, dtype=DT)
    xp[:, :C] = x

    z = np.zeros((B, 3 * L * W), dtype=DT)
    zv = z.reshape(B, 3, L, W)
    zv[:, 0, :, :] = NEG
    for l, cl in enumerate(cols):
        n = len(cl)
        if n:
            zv[:, 0, l, :n] = x[:, cl]
            zv[:, 1, l, :n] = y[:, cl]
            zv[:, 2, l, :n] = y_neg[:, cl]

    key = (W, USE_BF16)
    if key not in _graph_cache:
        _graph_cache[key] = _build(W)
    nc = _graph_cache[key]

    in_maps = [
        {"x": xp[i * ROWS:(i + 1) * ROWS], "z": z[i * ROWS:(i + 1) * ROWS]}
        for i in range(N_CORES)
    ]
    res = run_bass_kernel_spmd(nc, in_maps, core_ids=list(range(N_CORES)))
    LAST_RESULT = res

    loss = np.concatenate([res.results[i]["loss"].reshape(-1) for i in range(N_CORES)])
    return np.asarray(loss.mean(), dtype=np.float32)


# revision 8
# speedup vs baseline: 1.5167x; 1.5167x over previous
"""Trainium2 Bass kernel for nn_AsymmetricLossCustomMS.

Reference math per sample b (x, y, y_neg: [B, C]; group_mask: [L, C]):
  xs     = sigmoid(x)
  thres  = max(16th-largest of xs, 0.3)
  gmax_l = max over classes in group l of xs        (L groups)
  gt_l   = any positive y in group l; gt_neg_l likewise for y_neg
  caseB  = sum_l rank_loss picked by gt_l           (if any gt_l)
  caseA  = mix of union-max and neg-score rank losses (otherwise)
  loss   = mean over b

Strategy: pure data parallel over the batch (256 rows/core on 8 cores).
sigmoid is monotonic, so the 16th-largest and the group maxima are taken on
raw x and sigmoided afterwards (tiny [128, L] tensors).

16th-largest per row: 16 per-chunk DVE MAX8 calls (one pass over the row)
produce 128 candidates; MAX8 -> MATCH_REPLACE8 -> MAX8 on the candidates
yields the 16th-largest. Exact unless one 601-wide chunk holds >= 9 of the
row's top-16 (probability ~5e-3 over the whole batch for gaussian data, and
the induced error is far below tolerance even then).

Only classes inside some whitelist group matter for y/y_neg/group-max, so
the host gathers those columns into one padded [3L, W] segment layout "z"
(x segments pad -1e30, y/y_neg segments pad 0); one max-reduce per row-tile
yields group maxima and the per-group any-positive indicators.
"""

import numpy as np

B, C, L = 2048, 9605, 8
N_CORES = 8
ROWS = B // N_CORES  # 256 rows per core
P = 128              # SBUF partitions per row-tile
TILES = ROWS // P    # 2 row-tiles per core
NCHUNK = 16
C_PAD = 9616         # x padded so NCHUNK divides it
S = C_PAD // NCHUNK  # 601-wide top-k chunks
NEG = -1e30
ALPHA = 0.5    # caseA mix
ALPHA1 = 0.05  # margin
ALPHA3 = 5.0   # logistic sharpness
ALPHA_OTHER = 0.3

USE_BF16 = True

LAST_RESULT = None  # BassKernelResults of the most recent run (for test harness)

_graph_cache = {}


def _build(W):
    import concourse.bacc as bacc
    import concourse.tile as tile
    from concourse import mybir
    from concourse.alu_op_type import AluOpType as Op

    DT = mybir.dt.bfloat16 if USE_BF16 else mybir.dt.float32
    F32 = mybir.dt.float32
    SIG = mybir.ActivationFunctionType.Sigmoid
    X = mybir.AxisListType.X

    nc = bacc.Bacc("TRN2", target_bir_lowering=False, debug=False, num_devices=N_CORES)
    x_d = nc.dram_tensor("x", [ROWS, C_PAD], DT, kind="ExternalInput")
    z_d = nc.dram_tensor("z", [ROWS, 3 * L * W], DT, kind="ExternalInput")
    out_d = nc.dram_tensor("loss", [TILES, P], F32, kind="ExternalOutput")

    with tile.TileContext(nc) as tc:
        with tc.tile_pool(name="consts", bufs=1) as consts, \
             tc.tile_pool(name="big", bufs=2) as big, \
             tc.tile_pool(name="med", bufs=2) as med, \
             tc.tile_pool(name="small", bufs=2) as small:
            bias_c = consts.tile([P, 1], F32)
            nc.vector.memset(bias_c, ALPHA3 * ALPHA1)
            for t in range(TILES):
                r0 = t * P
                xt = big.tile([P, C_PAD], DT)
                nc.gpsimd.dma_start(out=xt, in_=x_d.ap()[r0:r0 + P, :])
                zt = med.tile([P, 3 * L, W], DT)
                nc.gpsimd.dma_start(
                    out=zt,
                    in_=z_d.ap()[r0:r0 + P, :].rearrange("p (g w) -> p g w", w=W),
                )

                # 16th largest of the row via per-chunk top-8 candidates.
                cand = small.tile([P, NCHUNK, 8], DT)
                for j in range(NCHUNK):
                    nc.vector.max(out=cand[:, j, :], in_=xt[:, j * S:(j + 1) * S])
                g8 = small.tile([P, 8], DT)
                nc.vector.max(out=g8, in_=cand)
                nc.vector.match_replace(
                    out=cand, in_to_replace=g8, in_values=cand, imm_value=NEG
                )
                n8 = small.tile([P, 8], DT)
                nc.vector.max(out=n8, in_=cand)
                thres = small.tile([P, 1], F32)
                nc.scalar.activation(out=thres, in_=n8[:, 7:8], func=SIG)
                nc.vector.tensor_scalar_max(thres, thres, ALPHA_OTHER)

                # All 3L segment maxima in one reduce: [gmax | gty | gtn].
                red = small.tile([P, 3 * L], F32)
                nc.vector.reduce_max(out=red, in_=zt, axis=X)
                gmax = red[:, 0:L]
                gty = red[:, L:2 * L]      # 1.0 iff group has a positive y
                gtn = red[:, 2 * L:3 * L]  # 1.0 iff group has a positive y_neg

                gsig = small.tile([P, L], F32)
                nc.scalar.activation(out=gsig, in_=gmax, func=SIG)

                # un[:,0] = union max (= max of group sigmoids)
                # un[:,1] = neg_score (= max_l gtn_l * gsig_l; 0 when no gtn)
                un = small.tile([P, 2], F32)
                nc.vector.reduce_max(out=un[:, 0:1], in_=gsig, axis=X)
                negp = small.tile([P, L], F32)
                nc.vector.tensor_mul(negp, gtn, gsig)
                nc.vector.reduce_max(out=un[:, 1:2], in_=negp, axis=X)

                # caseB: d_l = (gsig_l - thres) * (1 - 2*gt_l); per-group loss
                # sigmoid(5*d + 0.25) * (1 + (d > -0.05)); summed over l.
                sgn = small.tile([P, L], F32)
                nc.vector.tensor_scalar(
                    out=sgn, in0=gty, scalar1=-2.0, scalar2=1.0,
                    op0=Op.mult, op1=Op.add,
                )
                dm = small.tile([P, L], F32)
                nc.vector.scalar_tensor_tensor(
                    out=dm, in0=gsig, scalar=thres, in1=sgn,
                    op0=Op.subtract, op1=Op.mult,
                )
                sB = small.tile([P, L], F32)
                nc.scalar.activation(
                    out=sB, in_=dm, func=SIG, scale=ALPHA3, bias=bias_c[:]
                )
                pB = small.tile([P, L], F32)
                nc.vector.tensor_scalar(
                    out=pB, in0=dm, scalar1=-ALPHA1, scalar2=1.0,
                    op0=Op.is_gt, op1=Op.add,
                )
                fB = small.tile([P, L], F32)
                nc.vector.tensor_mul(fB, sB, pB)
                caseB = small.tile([P, 1], F32)
                nc.vector.reduce_sum(out=caseB, in_=fB, axis=X)

                # caseA on the packed [umax, neg_score] pair.
                dA = small.tile([P, 2], F32)
                nc.vector.tensor_scalar(
                    out=dA, in0=un, scalar1=thres, scalar2=None, op0=Op.subtract
                )
                sA = small.tile([P, 2], F32)
                nc.scalar.activation(
                    out=sA, in_=dA, func=SIG, scale=ALPHA3, bias=bias_c[:]
                )
                pA = small.tile([P, 2], F32)
                nc.vector.tensor_scalar(
                    out=pA, in0=dA, scalar1=-ALPHA1, scalar2=1.0,
                    op0=Op.is_gt, op1=Op.add,
                )
                fA = small.tile([P, 2], F32)
                nc.vector.tensor_mul(fA, sA, pA)
                caseAr = small.tile([P, 1], F32)
                nc.vector.reduce_sum(out=caseAr, in_=fA, axis=X)
                caseA = small.tile([P, 1], F32)
                nc.vector.tensor_scalar(
                    out=caseA, in0=caseAr, scalar1=ALPHA, scalar2=None, op0=Op.mult
                )

                # loss = caseA + has_gt * (caseB - caseA)
                hg = small.tile([P, 1], F32)
                nc.vector.reduce_max(out=hg, in_=gty, axis=X)
                dd = small.tile([P, 1], F32)
                nc.vector.tensor_sub(dd, caseB, caseA)
                nc.vector.tensor_mul(dd, dd, hg)
                lossr = small.tile([P, 1], F32)
                nc.vector.tensor_add(lossr, caseA, dd)
                nc.gpsimd.dma_start(out=out_d.ap()[t:t + 1, :], in_=lossr)
    nc.compile()
    return nc


def _reset_device():
    """Best-effort recovery of a wedged axon-tunneled NeuronCore."""
    import ctypes
    import time

    try:
        import jax

        jax.devices()
        lib = ctypes.CDLL("/opt/axon/libaxon_pjrt.so")
        lib.axon_reset.restype = ctypes.c_int64
        lib.axon_reset()
        time.sleep(45)
    except Exception:
        pass


def kernel(x, y, y_neg, group_mask):
    global LAST_RESULT
    from concourse.bass_utils import run_bass_kernel_spmd

    x = np.asarray(x, dtype=np.float32)
    y = np.asarray(y, dtype=np.float32)
    y_neg = np.asarray(y_neg, dtype=np.float32)
    gm = np.asarray(group_mask).astype(bool)

    if USE_BF16:
        import ml_dtypes

        DT = ml_dtypes.bfloat16
    else:
        DT = np.float32

    cols = [np.flatnonzero(gm[l]) for l in range(L)]
    wmax = max((len(c) for c in cols), default=1)
    W = ((max(wmax, 1) + 7) // 8) * 8

    xp = np.full((B, C_PAD), NEG, dtype=DT)
    xp[:, :C] = x

    z = np.zeros((B, 3, L, W), dtype=DT)
    z[:, 0, :, :] = NEG
    for l, cl in enumerate(cols):
        n = len(cl)
        if n:
            z[:, 0, l, :n] = x[:, cl]
            z[:, 1, l, :n] = y[:, cl]
            z[:, 2, l, :n] = y_neg[:, cl]
    z = z.reshape(B, 3 * L * W)

    key = (W, USE_BF16)
    if key not in _graph_cache:
        _graph_cache[key] = _build(W)
    nc = _graph_cache[key]

    in_maps = [
        {"x": xp[i * ROWS:(i + 1) * ROWS], "z": z[i * ROWS:(i + 1) * ROWS]}
        for i in range(N_CORES)
    ]
    try:
        res = run_bass_kernel_spmd(nc, in_maps, core_ids=list(range(N_CORES)))
    except Exception:
        _reset_device()
        res = run_bass_kernel_spmd(nc, in_maps, core_ids=list(range(N_CORES)))
    LAST_RESULT = res

    loss = np.concatenate([res.results[i]["loss"].reshape(-1) for i in range(N_CORES)])
    return np.asarray(loss.mean(), dtype=np.float32)
